# revision 10
# baseline (speedup 1.0000x reference)
"""Trainium2 Bass kernel for nn_DataEmbedding_cycle_pos.

Math (B=16, T=2048, N=8, D=512), out[b,t,:] =
    conv(x)               Conv1d(N->D, k=3, circular)        -> matmul K=24
  + temporal(x_mark)      sum of 4 fixed-table lookups; all indices < 7 and
                          the 4 tables share rows 0..6, so it's
                          onehot28 @ R4 (R4 = tile(R7, 4))    -> matmul K=28
  + cycle-positional      periods = clip(T/freq[argmax |rfft|], 1, T); for
                          T=2048 the period is 2048 unless the argmax is
                          exactly the Nyquist bin (then 1.0).  Per (b,n) only
                          the bit "is Nyquist the strict max" matters:
                            cyc[b] = (1-cnt/8)*postab + (cnt/8)*row01
                          cnt = #Nyquist-max series in batch b.
  The row01 (odd-column ones) term folds into the onehot matmul rows since
  sum(onehot) == 4 exactly:  R4 + (cnt/32)*odd.  The onehot rows are built on
  DVE as -onehot (min(|xm-v|-1, 0)) so the rhs rows carry -(R4 + bq*odd).
  The postab term accumulates into PSUM via an identity matmul with
  lhsT = (1-cnt/8)*I in bf16; the PSUM->SBUF copy alternates DVE/ACT.

Sharding: batch-parallel (2 batches/core).  The |rfft|^2 argmax test is
computed per core for its OWN 16 series with time-symmetric half DFT
tables (E/O fold, 9 chunks of t'=0..1024) — no collectives.
"""
import sys, os

sys.path.insert(0, "/opt/trn_rl_repo")
import numpy as np
import ml_dtypes

import concourse.bass as bass
import concourse.bacc as bacc
import concourse.mybir as mybir
import concourse.tile as tile
from concourse.bass_utils import run_bass_kernel_spmd

B, T, N, D = 16, 2048, 8, 512
NCORES = 8
BPC = B // NCORES          # batches per core
SPC = BPC * N              # series per core (16)
NT = T // 128              # 128-row time tiles per batch
KCONV = 3 * N              # 24 conv rows
KHOT = 28                  # 4 features x 7 index values
KTOT = KCONV + KHOT        # 52
NYQ = T // 2               # 1024
FTOT = 1032                # padded freq columns (1025 real, 7 zero pad)
FCH = (512, 512, 8)        # psum-bank-sized frequency chunks
FCHN = 9                   # t' chunks of 128 covering t'=0..1024

F32 = mybir.dt.float32
BF16 = mybir.dt.bfloat16
F32R = mybir.dt.float32r
BF = ml_dtypes.bfloat16

TRACE = False
TRACE_DIR = None

_cache = {}


# ----------------------------------------------------------------- constants
def _div_term():
    # mirror reference: exp(arange(0,512,2) * (-ln 10000 / 512)) in f32
    return np.exp(
        np.arange(0, D, 2, dtype=np.float32) * np.float32(-np.log(10000.0) / D)
    ).astype(np.float32)


def _fixed_rows(nrows):
    pos = np.arange(nrows, dtype=np.float32)[:, None]
    ang = (pos * _div_term()[None, :]).astype(np.float32)
    tab = np.zeros((nrows, D), dtype=np.float32)
    tab[:, 0::2] = np.sin(ang)
    tab[:, 1::2] = np.cos(ang)
    return tab


def _host_constants():
    c = {}
    postab = _fixed_rows(T)  # [2048, 512]
    # SBUF layout [128(tt), 16tiles * 512]
    c["postab"] = np.ascontiguousarray(
        postab.reshape(NT, 128, D).transpose(1, 0, 2).reshape(128, NT * D)
    ).astype(BF)
    r7 = _fixed_rows(7)
    odd = np.zeros((D,), dtype=np.float32)
    odd[1::2] = 1.0
    c["odd28"] = np.tile(odd[None, :], (KHOT, 1)).astype(np.float32)
    c["v28"] = np.tile(np.arange(7, dtype=np.float32), 4)[:, None].copy()
    c["r4n"] = np.ascontiguousarray(-np.tile(r7, (4, 1))).astype(np.float32)

    # symmetric-half DFT tables: Re uses E[t']=x_t'+x_{2048-t'} against
    # cos rows t'=0..1024 (rows 0,1024 halved since E doubles them);
    # Im uses O[t']=x_t'-x_{2048-t'} against sin rows (0,1024 naturally 0).
    # Layout [128(tt), 9 ch * FTOT], rows beyond t'=1024 zero.
    tp = np.arange(FCHN * 128, dtype=np.float64)               # t' padded
    k_idx = np.arange(FTOT, dtype=np.float64)
    kmask = (k_idx <= NYQ).astype(np.float64)
    tmask = (tp <= NYQ).astype(np.float64)
    half = np.where((tp == 0) | (tp == NYQ), 0.5, 1.0)
    ang = 2.0 * np.pi * np.outer(tp, k_idx) / T                # [1152, 1032]
    cm = (np.cos(ang) * kmask[None, :] * (tmask * half)[:, None]).astype(np.float32)
    sm = (np.sin(ang) * kmask[None, :] * tmask[:, None]).astype(np.float32)
    c["cdft"] = np.ascontiguousarray(
        cm.reshape(FCHN, 128, FTOT).transpose(1, 0, 2).reshape(128, FCHN * FTOT)
    ).astype(BF)
    c["sdft"] = np.ascontiguousarray(
        sm.reshape(FCHN, 128, FTOT).transpose(1, 0, 2).reshape(128, FCHN * FTOT)
    ).astype(BF)
    c["i128"] = np.eye(128, dtype=np.float32).astype(BF)

    # batch indicator for the cnt matmul: ind2[s, i*128+p] = (s//8 == i)
    s_batch = np.arange(SPC) // N
    cols = [np.tile((s_batch == i).astype(np.float32)[:, None], (1, 128))
            for i in range(BPC)]
    c["ind2"] = np.concatenate(cols, axis=1).astype(BF)        # [16, 256]
    return c


# ------------------------------------------------------------------- program
def _build_nc():
    nc = bacc.Bacc("TRN2", target_bir_lowering=False, debug=False,
                   num_devices=NCORES)

    def din(name, shape, dt):
        return nc.dram_tensor(name, shape, dt, kind="ExternalInput").ap()

    xtp = din("xtp", [BPC, N, T + 2], F32R)       # circular-padded x^T
    xm7 = din("xm7", [BPC, KHOT, T], F32)         # x_mark rows repeated 7x
    fwd = din("fwd", [128, FCHN * SPC], BF16)     # x[t'] per (ch, s)
    bwd = din("bwd", [128, FCHN * SPC], BF16)     # x[(2048-t') % 2048]
    cdft = din("cdft", [128, FCHN * FTOT], BF16)
    sdft = din("sdft", [128, FCHN * FTOT], BF16)
    postab = din("postab", [128, NT * D], BF16)
    w24 = din("w24", [KCONV, D], F32R)
    r4n = din("r4n", [KHOT, D], F32)
    odd28 = din("odd28", [KHOT, D], F32)
    v28 = din("v28", [KHOT, 1], F32)
    i128 = din("i128", [128, 128], BF16)
    ind2 = din("ind2", [SPC, BPC * 128], BF16)
    out = nc.dram_tensor("out", [BPC, T, D], F32, kind="ExternalOutput").ap()

    with tile.TileContext(nc) as tc:
        with (
            tc.tile_pool(name="consts", bufs=1) as cpool,
            tc.tile_pool(name="fwork", bufs=1) as fpool,
            tc.tile_pool(name="fpsum", bufs=1, space="PSUM") as fpsum,
            tc.tile_pool(name="cpsum", bufs=1, space="PSUM") as cpsum,
            tc.tile_pool(name="mpsum", bufs=5, space="PSUM") as mpsum,
            tc.tile_pool(name="batch", bufs=2) as bpool,
            tc.tile_pool(name="outp", bufs=4) as opool,
        ):
            # ---------------- FFT phase: own-series |rfft|^2 over all bins
            fwd_sb = fpool.tile([128, FCHN * SPC], BF16, tag="fwd")
            nc.sync.dma_start(fwd_sb[:], fwd)
            bwd_sb = fpool.tile([128, FCHN * SPC], BF16, tag="bwd")
            nc.sync.dma_start(bwd_sb[:], bwd)
            ee = fpool.tile([128, FCHN * SPC], BF16, tag="ee")
            nc.vector.tensor_add(ee[:], fwd_sb[:], bwd_sb[:])
            oo = fpool.tile([128, FCHN * SPC], BF16, tag="oo")
            nc.vector.tensor_sub(oo[:], fwd_sb[:], bwd_sb[:])

            # per-chunk table tiles so matmuls start as chunks land
            cch, sch = [], []
            for ch in range(FCHN):
                ct = cpool.tile([128, FTOT], BF16, tag=f"cd{ch}")
                nc.sync.dma_start(ct[:], cdft[:, ch * FTOT:(ch + 1) * FTOT])
                cch.append(ct)
                st = cpool.tile([128, FTOT], BF16, tag=f"sd{ch}")
                nc.sync.dma_start(st[:], sdft[:, ch * FTOT:(ch + 1) * FTOT])
                sch.append(st)

            mag = fpool.tile([SPC, FTOT], F32, tag="mag")
            sq = fpool.tile([SPC, 512], F32, tag="sq")
            off = 0
            for fc, fw in enumerate(FCH):
                ps_re = fpsum.tile([SPC, fw], F32, tag="psre")
                ps_im = fpsum.tile([SPC, fw], F32, tag="psim")
                for ch in range(FCHN):
                    nc.tensor.matmul(
                        ps_re[:], ee[:, ch * SPC:(ch + 1) * SPC],
                        cch[ch][:, off:off + fw],
                        start=(ch == 0), stop=(ch == FCHN - 1))
                    nc.tensor.matmul(
                        ps_im[:], oo[:, ch * SPC:(ch + 1) * SPC],
                        sch[ch][:, off:off + fw],
                        start=(ch == 0), stop=(ch == FCHN - 1))
                nc.scalar.square(mag[:, off:off + fw], ps_re[:])
                nc.scalar.square(sq[:, 0:fw], ps_im[:])
                nc.vector.tensor_add(mag[:, off:off + fw],
                                     mag[:, off:off + fw], sq[:, 0:fw])
                off += fw

            # strict >: Nyquist wins only if greater than every earlier bin
            lmax = fpool.tile([SPC, 1], F32, tag="lmax")
            nc.vector.reduce_max(lmax[:], mag[:, 0:NYQ],
                                 axis=mybir.AxisListType.X)
            isn = fpool.tile([SPC, 1], BF16, tag="isn")
            nc.vector.tensor_tensor(isn[:], mag[:, NYQ:NYQ + 1], lmax[:],
                                    op=mybir.AluOpType.is_gt)

            ind2_sb = cpool.tile([SPC, BPC * 128], BF16, tag="ind2")
            nc.sync.dma_start(ind2_sb[:], ind2)
            i128_sb = cpool.tile([128, 128], BF16, tag="i128")
            nc.sync.dma_start(i128_sb[:], i128)

            a_is, bqn_vecs = [], []
            for i in range(BPC):
                ps_cnt = cpsum.tile([128, 1], F32, tag="pscnt")
                nc.tensor.matmul(ps_cnt[:], ind2_sb[:, i * 128:(i + 1) * 128],
                                 isn[:], start=True, stop=True)
                a_vec = fpool.tile([128, 1], F32, tag=f"avec{i}")
                nc.vector.tensor_scalar(a_vec[:], ps_cnt[:], -0.125, 1.0,
                                        op0=mybir.AluOpType.mult,
                                        op1=mybir.AluOpType.add)
                ai = fpool.tile([128, 128], BF16, tag=f"ai{i}")
                nc.vector.tensor_scalar(ai[:], i128_sb[:], a_vec[:], None,
                                        op0=mybir.AluOpType.mult)
                bqn_vec = fpool.tile([128, 1], F32, tag=f"bqvec{i}")
                nc.vector.tensor_scalar(bqn_vec[:], ps_cnt[:], -1.0 / 32.0,
                                        None, op0=mybir.AluOpType.mult)
                a_is.append(ai)
                bqn_vecs.append(bqn_vec)

            # ---------------- constants for the main matmul
            postab_sb = cpool.tile([128, NT * D], BF16, tag="postab")
            nc.sync.dma_start(postab_sb[:], postab)
            r4n_sb = cpool.tile([KHOT, D], F32, tag="r4n")
            nc.sync.dma_start(r4n_sb[:], r4n)
            odd28_sb = cpool.tile([KHOT, D], F32, tag="odd28")
            nc.sync.dma_start(odd28_sb[:], odd28)
            v28_sb = cpool.tile([KHOT, 1], F32, tag="v28")
            nc.sync.dma_start(v28_sb[:], v28)

            # ---------------- main per-batch pipelines
            # lt row layout: [0:28] -onehot (DVE, base partition 0),
            #                [28:52] conv x rows (DMA, any base legal)
            for i in range(BPC):
                lt = bpool.tile([KTOT, T], F32R, tag="lt")
                for k in range(3):
                    nc.sync.dma_start(lt[KHOT + k * N:KHOT + (k + 1) * N, :],
                                      xtp[i, :, k:k + T])
                xm = bpool.tile([KHOT, T], F32, tag="xm")
                nc.sync.dma_start(xm[:], xm7[i])
                t28 = bpool.tile([KHOT, T], F32, tag="t28")
                # t28 = |xm - v| (ACT);  lt[0:28] = min(t28 - 1, 0) = -onehot
                nc.scalar.activation(t28[:], xm[:],
                                     mybir.ActivationFunctionType.Abs,
                                     bias=v28_sb[:], scale=-1.0)
                nc.vector.tensor_scalar(lt[0:KHOT, :], t28[:], 1.0, 0.0,
                                        op0=mybir.AluOpType.subtract,
                                        op1=mybir.AluOpType.min)

                rhs = bpool.tile([KTOT, D], F32R, tag="rhs")
                nc.sync.dma_start(rhs[KHOT:KTOT, :], w24)
                # -(R4 + (cnt/32)*odd): sum(onehot)==4 folds the odd term
                nc.vector.scalar_tensor_tensor(
                    rhs[0:KHOT, :], odd28_sb[:], bqn_vecs[i][0:KHOT, :],
                    r4n_sb[:], op0=mybir.AluOpType.mult, op1=mybir.AluOpType.add)

                for ti in range(NT):
                    ps = mpsum.tile([128, D], F32, tag="ps")
                    nc.tensor.matmul(ps[:],
                                     lt[:, ti * 128:(ti + 1) * 128],
                                     rhs[:],
                                     start=True, stop=False)
                    nc.tensor.matmul(ps[:], a_is[i][:],
                                     postab_sb[:, ti * D:(ti + 1) * D],
                                     start=False, stop=True)
                    ot = opool.tile([128, D], F32, tag="ot")
                    if ti % 2 == 0:
                        nc.vector.tensor_copy(ot[:], ps[:])
                    else:
                        nc.scalar.copy(ot[:], ps[:])
                    nc.sync.dma_start(out[i, ti * 128:(ti + 1) * 128, :], ot[:])
    nc.compile()
    return nc


def _get_nc():
    if "nc" not in _cache:
        _cache["nc"] = _build_nc()
    return _cache["nc"]


def _host_inputs(x, x_mark, conv_w):
    # x^T with circular pad: xtp[b, n, j] = x[b, (j-1) % T, n]
    xt = np.ascontiguousarray(x.transpose(0, 2, 1))        # [16, 8, 2048]
    xtp = np.concatenate([xt[:, :, -1:], xt, xt[:, :, :1]], axis=2)
    # x_mark as f32, transposed, each feature row repeated 7x -> [16, 28, T]
    xmt = x_mark.astype(np.float32).transpose(0, 2, 1)     # [16, 4, 2048]
    xm7 = np.repeat(xmt, 7, axis=1)                        # [16, 28, 2048]
    # per-core fwd/bwd FFT operands [tt, ch*16 + s], s = b_loc*8 + n,
    # t' = ch*128+tt valid through 1024, zero beyond
    tp = np.arange(FCHN * 128)
    valid = tp <= NYQ
    fwd_idx = np.where(valid, tp, 0)
    bwd_idx = np.where(valid, (T - tp) % T, 0)
    fwds, bwds = [], []
    for core in range(NCORES):
        xs = x[core * BPC:(core + 1) * BPC]                # [2, 2048, 8]
        xflat = xs.transpose(1, 0, 2).reshape(T, SPC)      # [t, s]
        fw_ = xflat[fwd_idx] * valid[:, None]              # [1152, 16]
        bw_ = xflat[bwd_idx] * valid[:, None]
        fwds.append(np.ascontiguousarray(
            fw_.reshape(FCHN, 128, SPC).transpose(1, 0, 2)
               .reshape(128, FCHN * SPC)).astype(BF))
        bwds.append(np.ascontiguousarray(
            bw_.reshape(FCHN, 128, SPC).transpose(1, 0, 2)
               .reshape(128, FCHN * SPC)).astype(BF))
    # conv weight rows (k, n): w24[k*8+n, d] = conv_w[d, n, k]
    w24 = np.ascontiguousarray(conv_w.transpose(2, 1, 0).reshape(KCONV, D))
    return xtp, xm7, fwds, bwds, w24


def make_in_maps(x, x_mark, conv_w):
    if "consts" not in _cache:
        _cache["consts"] = _host_constants()
    c = _cache["consts"]
    xtp, xm7, fwds, bwds, w24 = _host_inputs(x, x_mark, conv_w)
    in_maps = []
    for core in range(NCORES):
        b0 = core * BPC
        in_maps.append({
            "xtp": np.ascontiguousarray(xtp[b0:b0 + BPC]),
            "xm7": np.ascontiguousarray(xm7[b0:b0 + BPC]),
            "fwd": fwds[core],
            "bwd": bwds[core],
            "cdft": c["cdft"],
            "sdft": c["sdft"],
            "postab": c["postab"],
            "w24": w24.astype(np.float32),
            "r4n": c["r4n"],
            "odd28": c["odd28"],
            "v28": c["v28"],
            "i128": c["i128"],
            "ind2": c["ind2"],
        })
    return in_maps


# -------------------------------------------------------------------- driver
def kernel(**inputs):
    x = np.asarray(inputs["x"], dtype=np.float32)          # [16, 2048, 8]
    x_mark = np.asarray(inputs["x_mark"])                  # [16, 2048, 4] int
    conv_w = np.asarray(inputs["conv_w"], dtype=np.float32)  # [512, 8, 3]

    in_maps = make_in_maps(x, x_mark, conv_w)
    nc = _get_nc()
    kw = {}
    if TRACE:
        kw = dict(trace=True, tmpdir=TRACE_DIR)
    br = run_bass_kernel_spmd(nc, in_maps, list(range(NCORES)), **kw)
    if TRACE:
        _cache["last_results"] = br

    outp = np.empty((B, T, D), dtype=np.float32)
    for core in range(NCORES):
        outp[core * BPC:(core + 1) * BPC] = br.results[core]["out"]
    return outp


# revision 11
# speedup vs baseline: 1.1114x; 1.1114x over previous
"""Trainium2 Bass kernel for nn_DataEmbedding_cycle_pos.

Math (B=16, T=2048, N=8, D=512), out[b,t,:] =
    conv(x)               Conv1d(N->D, k=3, circular)        -> matmul K=24
  + temporal(x_mark)      sum of 4 fixed-table lookups; all indices < 7 and
                          the 4 tables share rows 0..6, so it's
                          onehot28 @ R4 (R4 = tile(R7, 4))    -> matmul K=28
  + cycle-positional      periods = clip(T/freq[argmax |rfft|], 1, T); for
                          T=2048 the period is 2048 unless the argmax is
                          exactly the Nyquist bin (then 1.0).  Per (b,n) only
                          the bit "is Nyquist the strict max" matters:
                            cyc[b] = (1-cnt/8)*postab + (cnt/8)*row01
                          cnt = #Nyquist-max series in batch b.
  The row01 (odd-column ones) term folds into the onehot matmul rows since
  sum(onehot) == 4 exactly:  R4 + (cnt/32)*odd.  The onehot rows are built on
  DVE as -onehot (min(|xm-v|-1, 0)) so the rhs rows carry -(R4 + bq*odd).
  The postab term accumulates into PSUM via an identity matmul with
  lhsT = (1-cnt/8)*I in bf16; the PSUM->SBUF copy alternates DVE/ACT.

Sharding: batch-parallel (2 batches/core).  The |rfft|^2 argmax test is
computed per core for its OWN 16 series with time-symmetric half DFT
tables (E/O fold, 9 chunks of t'=0..1024) — no collectives.
"""
import sys, os

sys.path.insert(0, "/opt/trn_rl_repo")
import numpy as np
import ml_dtypes

import concourse.bass as bass
import concourse.bacc as bacc
import concourse.mybir as mybir
import concourse.tile as tile
from concourse.bass_utils import run_bass_kernel_spmd

B, T, N, D = 16, 2048, 8, 512
NCORES = 8
BPC = B // NCORES          # batches per core
SPC = BPC * N              # series per core (16)
NT = T // 128              # 128-row time tiles per batch
KCONV = 3 * N              # 24 conv rows
KHOT = 28                  # 4 features x 7 index values
KTOT = KCONV + KHOT        # 52
NYQ = T // 2               # 1024
FTOT = 1032                # padded freq columns (1025 real, 7 zero pad)
FCH = (512, 512, 8)        # psum-bank-sized frequency chunks
FCHN = 9                   # t' chunks of 128 covering t'=0..1024

F32 = mybir.dt.float32
BF16 = mybir.dt.bfloat16
F32R = mybir.dt.float32r
BF = ml_dtypes.bfloat16

TRACE = False
TRACE_DIR = None

_cache = {}


# ----------------------------------------------------------------- constants
def _div_term():
    # mirror reference: exp(arange(0,512,2) * (-ln 10000 / 512)) in f32
    return np.exp(
        np.arange(0, D, 2, dtype=np.float32) * np.float32(-np.log(10000.0) / D)
    ).astype(np.float32)


def _fixed_rows(nrows):
    pos = np.arange(nrows, dtype=np.float32)[:, None]
    ang = (pos * _div_term()[None, :]).astype(np.float32)
    tab = np.zeros((nrows, D), dtype=np.float32)
    tab[:, 0::2] = np.sin(ang)
    tab[:, 1::2] = np.cos(ang)
    return tab


def _host_constants():
    c = {}
    postab = _fixed_rows(T)  # [2048, 512]
    # SBUF layout [128(tt), 16tiles * 512]
    c["postab"] = np.ascontiguousarray(
        postab.reshape(NT, 128, D).transpose(1, 0, 2).reshape(128, NT * D)
    ).astype(BF)
    r7 = _fixed_rows(7)
    odd = np.zeros((D,), dtype=np.float32)
    odd[1::2] = 1.0
    c["odd28"] = np.tile(odd[None, :], (KHOT, 1)).astype(np.float32)
    c["v28"] = np.tile(np.arange(7, dtype=np.float32), 4)[:, None].copy()
    c["r4"] = np.ascontiguousarray(np.tile(r7, (4, 1))).astype(np.float32)

    # symmetric-half DFT tables: Re uses E[t']=x_t'+x_{2048-t'} against
    # cos rows t'=0..1024 (rows 0,1024 halved since E doubles them);
    # Im uses O[t']=x_t'-x_{2048-t'} against sin rows (0,1024 naturally 0).
    # Layout [128(tt), 9 ch * FTOT], rows beyond t'=1024 zero.
    tp = np.arange(FCHN * 128, dtype=np.float64)               # t' padded
    k_idx = np.arange(FTOT, dtype=np.float64)
    kmask = (k_idx <= NYQ).astype(np.float64)
    tmask = (tp <= NYQ).astype(np.float64)
    half = np.where((tp == 0) | (tp == NYQ), 0.5, 1.0)
    ang = 2.0 * np.pi * np.outer(tp, k_idx) / T                # [1152, 1032]
    cm = (np.cos(ang) * kmask[None, :] * (tmask * half)[:, None]).astype(np.float32)
    sm = (np.sin(ang) * kmask[None, :] * tmask[:, None]).astype(np.float32)
    c["cdft"] = np.ascontiguousarray(
        cm.reshape(FCHN, 128, FTOT).transpose(1, 0, 2).reshape(128, FCHN * FTOT)
    ).astype(BF)
    c["sdft"] = np.ascontiguousarray(
        sm.reshape(FCHN, 128, FTOT).transpose(1, 0, 2).reshape(128, FCHN * FTOT)
    ).astype(BF)

    # batch indicator for the cnt matmul: ind2[s, i*128+p] = (s//8 == i)
    s_batch = np.arange(SPC) // N
    cols = [np.tile((s_batch == i).astype(np.float32)[:, None], (1, 128))
            for i in range(BPC)]
    c["ind2"] = np.concatenate(cols, axis=1).astype(BF)        # [16, 256]
    return c


# ------------------------------------------------------------------- program
def _build_nc():
    nc = bacc.Bacc("TRN2", target_bir_lowering=False, debug=False,
                   num_devices=NCORES)

    def din(name, shape, dt):
        return nc.dram_tensor(name, shape, dt, kind="ExternalInput").ap()

    xtp = din("xtp", [BPC, N, T + 2], F32R)       # circular-padded x^T
    xm7 = din("xm7", [BPC, KHOT, T], F32)         # x_mark rows repeated 7x
    fwd = din("fwd", [128, FCHN * SPC], BF16)     # x[t'] per (ch, s)
    bwd = din("bwd", [128, FCHN * SPC], BF16)     # x[(2048-t') % 2048]
    cdft = din("cdft", [128, FCHN * FTOT], BF16)
    sdft = din("sdft", [128, FCHN * FTOT], BF16)
    postab = din("postab", [128, NT * D], BF16)
    w24 = din("w24", [KCONV, D], F32R)
    r4 = din("r4", [KHOT, D], F32)
    odd28 = din("odd28", [KHOT, D], F32)
    v28 = din("v28", [KHOT, 1], F32)
    ind2 = din("ind2", [SPC, BPC * 128], BF16)
    out = nc.dram_tensor("out", [BPC, T, D], F32, kind="ExternalOutput").ap()

    with tile.TileContext(nc) as tc:
        with (
            tc.tile_pool(name="consts", bufs=1) as cpool,
            tc.tile_pool(name="fwork", bufs=1) as fpool,
            tc.tile_pool(name="fpsum", bufs=1, space="PSUM") as fpsum,
            tc.tile_pool(name="cpsum", bufs=1, space="PSUM") as cpsum,
            tc.tile_pool(name="mpsum", bufs=5, space="PSUM") as mpsum,
            tc.tile_pool(name="batch", bufs=2) as bpool,
            tc.tile_pool(name="outp", bufs=6) as opool,
        ):
            # ---------------- FFT phase: own-series |rfft|^2 over all bins
            fwd_sb = fpool.tile([128, FCHN * SPC], BF16, tag="fwd")
            nc.sync.dma_start(fwd_sb[:], fwd)
            bwd_sb = fpool.tile([128, FCHN * SPC], BF16, tag="bwd")
            nc.sync.dma_start(bwd_sb[:], bwd)
            ee = fpool.tile([128, FCHN * SPC], BF16, tag="ee")
            nc.vector.tensor_add(ee[:], fwd_sb[:], bwd_sb[:])
            oo = fpool.tile([128, FCHN * SPC], BF16, tag="oo")
            nc.vector.tensor_sub(oo[:], fwd_sb[:], bwd_sb[:])

            # per-chunk table tiles so matmuls start as chunks land
            cch, sch = [], []
            for ch in range(FCHN):
                ct = cpool.tile([128, FTOT], BF16, tag=f"cd{ch}")
                nc.sync.dma_start(ct[:], cdft[:, ch * FTOT:(ch + 1) * FTOT])
                cch.append(ct)
                st = cpool.tile([128, FTOT], BF16, tag=f"sd{ch}")
                nc.sync.dma_start(st[:], sdft[:, ch * FTOT:(ch + 1) * FTOT])
                sch.append(st)

            mag = fpool.tile([SPC, FTOT], F32, tag="mag")
            sq = fpool.tile([SPC, 512], F32, tag="sq")
            off = 0
            for fc, fw in enumerate(FCH):
                ps_re = fpsum.tile([SPC, fw], F32, tag="psre")
                ps_im = fpsum.tile([SPC, fw], F32, tag="psim")
                for ch in range(FCHN):
                    nc.tensor.matmul(
                        ps_re[:], ee[:, ch * SPC:(ch + 1) * SPC],
                        cch[ch][:, off:off + fw],
                        start=(ch == 0), stop=(ch == FCHN - 1))
                    nc.tensor.matmul(
                        ps_im[:], oo[:, ch * SPC:(ch + 1) * SPC],
                        sch[ch][:, off:off + fw],
                        start=(ch == 0), stop=(ch == FCHN - 1))
                nc.scalar.square(mag[:, off:off + fw], ps_re[:])
                nc.scalar.square(sq[:, 0:fw], ps_im[:])
                nc.gpsimd.tensor_add(mag[:, off:off + fw],
                                      mag[:, off:off + fw], sq[:, 0:fw])
                off += fw

            # strict >: Nyquist wins only if greater than every earlier bin
            lmax = fpool.tile([SPC, 1], F32, tag="lmax")
            nc.vector.reduce_max(lmax[:], mag[:, 0:NYQ],
                                 axis=mybir.AxisListType.X)
            isn = fpool.tile([SPC, 1], BF16, tag="isn")
            nc.vector.tensor_tensor(isn[:], mag[:, NYQ:NYQ + 1], lmax[:],
                                    op=mybir.AluOpType.is_gt)

            ind2_sb = cpool.tile([SPC, BPC * 128], BF16, tag="ind2")
            nc.sync.dma_start(ind2_sb[:], ind2)

            a_vecs, bq_vecs = [], []
            for i in range(BPC):
                ps_cnt = cpsum.tile([128, 1], F32, tag="pscnt")
                nc.tensor.matmul(ps_cnt[:], ind2_sb[:, i * 128:(i + 1) * 128],
                                 isn[:], start=True, stop=True)
                a_vec = fpool.tile([128, 1], F32, tag=f"avec{i}")
                nc.vector.tensor_scalar(a_vec[:], ps_cnt[:], -0.125, 1.0,
                                        op0=mybir.AluOpType.mult,
                                        op1=mybir.AluOpType.add)
                bq_vec = fpool.tile([128, 1], F32, tag=f"bqvec{i}")
                nc.vector.tensor_scalar(bq_vec[:], ps_cnt[:], 1.0 / 32.0,
                                        None, op0=mybir.AluOpType.mult)
                a_vecs.append(a_vec)
                bq_vecs.append(bq_vec)

            # ---------------- constants for the main matmul
            postab_sb = cpool.tile([128, NT * D], BF16, tag="postab")
            nc.sync.dma_start(postab_sb[:], postab)
            r4_sb = cpool.tile([KHOT, D], F32, tag="r4")
            nc.sync.dma_start(r4_sb[:], r4)
            odd28_sb = cpool.tile([KHOT, D], F32, tag="odd28")
            nc.sync.dma_start(odd28_sb[:], odd28)
            v28_sb = cpool.tile([KHOT, 1], F32, tag="v28")
            nc.sync.dma_start(v28_sb[:], v28)

            # ---------------- main per-batch pipelines
            # lt row layout: [0:28] -onehot (DVE, base partition 0),
            #                [28:52] conv x rows (DMA, any base legal)
            for i in range(BPC):
                lt = bpool.tile([KTOT, T], F32R, tag="lt")
                for k in range(3):
                    nc.sync.dma_start(lt[KHOT + k * N:KHOT + (k + 1) * N, :],
                                      xtp[i, :, k:k + T])
                xm = bpool.tile([KHOT, T], F32, tag="xm")
                nc.sync.dma_start(xm[:], xm7[i])
                t28 = bpool.tile([KHOT, T], F32, tag="t28")
                # t28 = |xm - v|;  lt[0:28] = relu(1 - t28) = onehot
                nc.scalar.activation(t28[:], xm[:],
                                     mybir.ActivationFunctionType.Abs,
                                     bias=v28_sb[:], scale=-1.0)
                nc.scalar.activation(lt[0:KHOT, :], t28[:],
                                     mybir.ActivationFunctionType.Relu,
                                     bias=1.0, scale=-1.0)

                rhs = bpool.tile([KTOT, D], F32R, tag="rhs")
                nc.sync.dma_start(rhs[KHOT:KTOT, :], w24)
                # R4 + (cnt/32)*odd: sum(onehot)==4 folds the odd term
                nc.vector.scalar_tensor_tensor(
                    rhs[0:KHOT, :], odd28_sb[:], bq_vecs[i][0:KHOT, :],
                    r4_sb[:], op0=mybir.AluOpType.mult, op1=mybir.AluOpType.add)

                for ti in range(NT):
                    ps = mpsum.tile([128, D], F32, tag="ps")
                    nc.tensor.matmul(ps[:],
                                     lt[:, ti * 128:(ti + 1) * 128],
                                     rhs[:],
                                     start=True, stop=True)
                    ot = opool.tile([128, D], F32, tag="ot")
                    nc.vector.scalar_tensor_tensor(
                        ot[:], postab_sb[:, ti * D:(ti + 1) * D], a_vecs[i][:],
                        ps[:], op0=mybir.AluOpType.mult, op1=mybir.AluOpType.add)
                    nc.sync.dma_start(out[i, ti * 128:(ti + 1) * 128, :], ot[:])
    nc.compile()
    return nc


def _get_nc():
    if "nc" not in _cache:
        _cache["nc"] = _build_nc()
    return _cache["nc"]


def _host_inputs(x, x_mark, conv_w):
    # x^T with circular pad: xtp[b, n, j] = x[b, (j-1) % T, n]
    xt = np.ascontiguousarray(x.transpose(0, 2, 1))        # [16, 8, 2048]
    xtp = np.concatenate([xt[:, :, -1:], xt, xt[:, :, :1]], axis=2)
    # x_mark as f32, transposed, each feature row repeated 7x -> [16, 28, T]
    xmt = x_mark.astype(np.float32).transpose(0, 2, 1)     # [16, 4, 2048]
    xm7 = np.repeat(xmt, 7, axis=1)                        # [16, 28, 2048]
    # per-core fwd/bwd FFT operands [tt, ch*16 + s], s = b_loc*8 + n,
    # t' = ch*128+tt valid through 1024, zero beyond
    tp = np.arange(FCHN * 128)
    valid = tp <= NYQ
    fwd_idx = np.where(valid, tp, 0)
    bwd_idx = np.where(valid, (T - tp) % T, 0)
    fwds, bwds = [], []
    for core in range(NCORES):
        xs = x[core * BPC:(core + 1) * BPC]                # [2, 2048, 8]
        xflat = xs.transpose(1, 0, 2).reshape(T, SPC)      # [t, s]
        fw_ = xflat[fwd_idx] * valid[:, None]              # [1152, 16]
        bw_ = xflat[bwd_idx] * valid[:, None]
        fwds.append(np.ascontiguousarray(
            fw_.reshape(FCHN, 128, SPC).transpose(1, 0, 2)
               .reshape(128, FCHN * SPC)).astype(BF))
        bwds.append(np.ascontiguousarray(
            bw_.reshape(FCHN, 128, SPC).transpose(1, 0, 2)
               .reshape(128, FCHN * SPC)).astype(BF))
    # conv weight rows (k, n): w24[k*8+n, d] = conv_w[d, n, k]
    w24 = np.ascontiguousarray(conv_w.transpose(2, 1, 0).reshape(KCONV, D))
    return xtp, xm7, fwds, bwds, w24


def make_in_maps(x, x_mark, conv_w):
    if "consts" not in _cache:
        _cache["consts"] = _host_constants()
    c = _cache["consts"]
    xtp, xm7, fwds, bwds, w24 = _host_inputs(x, x_mark, conv_w)
    in_maps = []
    for core in range(NCORES):
        b0 = core * BPC
        in_maps.append({
            "xtp": np.ascontiguousarray(xtp[b0:b0 + BPC]),
            "xm7": np.ascontiguousarray(xm7[b0:b0 + BPC]),
            "fwd": fwds[core],
            "bwd": bwds[core],
            "cdft": c["cdft"],
            "sdft": c["sdft"],
            "postab": c["postab"],
            "w24": w24.astype(np.float32),
            "r4": c["r4"],
            "odd28": c["odd28"],
            "v28": c["v28"],
            "ind2": c["ind2"],
        })
    return in_maps


# -------------------------------------------------------------------- driver
def kernel(**inputs):
    x = np.asarray(inputs["x"], dtype=np.float32)          # [16, 2048, 8]
    x_mark = np.asarray(inputs["x_mark"])                  # [16, 2048, 4] int
    conv_w = np.asarray(inputs["conv_w"], dtype=np.float32)  # [512, 8, 3]

    in_maps = make_in_maps(x, x_mark, conv_w)
    nc = _get_nc()
    kw = {}
    if TRACE:
        kw = dict(trace=True, tmpdir=TRACE_DIR)
    br = run_bass_kernel_spmd(nc, in_maps, list(range(NCORES)), **kw)
    if TRACE:
        _cache["last_results"] = br

    outp = np.empty((B, T, D), dtype=np.float32)
    for core in range(NCORES):
        outp[core * BPC:(core + 1) * BPC] = br.results[core]["out"]
    return outp
